# revision 31
# baseline (speedup 1.0000x reference)
"""Gated Linear Attention forward on 8 Trainium2 NeuronCores (Bass/Tile).

Problem: B=4, T=1024, D=1024, H=8, DK=64, DV=128, conv4 on q/k/v, low-rank
log-sigmoid forget gate, recurrent scan, RMS-norm + swish output gate, out proj.

Sharding: core = 2*b + hg  (b = batch, hg = half of the heads).
Each core computes its batch's tokens for 4 heads end-to-end and a partial
output projection (Wo row-block); the host sums the two partials per batch.

On-device algorithm: chunked-parallel GLA with chunk C=128.
Per chunk (local inclusive cumsum b of the log-gates):
  q~ = q * exp(b)/8,  k~ = k * exp(-b),  k^ = k~ * exp(b_C)
  A~[s,t] = sum_kk k~[s] q~[t]   masked to s<=t
  o = A~^T v (intra) + q~ @ S (inter), accumulated in one PSUM tile
  S' = diag(exp(b_C)) S + k^T v

v3 scheme:
- src/weights arrive bf16, host-packed contiguous; GLA core math stays
  f32r/fp32.  Projection/conv/gate matmuls are bf16 (FWL weight loads).
- DMA queues are chained need-ordered (src + gk weights first) because HBM
  arbitration is round-robin across queues: a late-needed weight on its own
  queue steals head bandwidth from src.
- ACT tables: exp/ln family first (gk softplus chain, texp, Eall), then one
  switch into the silu set for all conv/gate evacuations; the RMS-norm rsqrt
  is computed on the DVE (bit-hack seed + 2 Newton steps) so the tail needs
  no table switch at all.
- The chunk recurrence (DVE/ACT-paced) is interleaved with the v-conv and
  gate projections (PE-paced) in two halves, so the serial chunk chain hides
  under dense PE work instead of running after it.
"""

import numpy as np
import ml_dtypes

import concourse.bass as bass
import concourse.mybir as mybir
import concourse.tile as tile
from concourse import bacc
from concourse.bass_utils import run_bass_kernel_spmd

F32 = mybir.dt.float32
F32R = mybir.dt.float32r
BF16 = mybir.dt.bfloat16
I32 = mybir.dt.int32
AF = mybir.ActivationFunctionType
OP = mybir.AluOpType

# problem constants (hardcoded per the task contract)
B, T, D, H = 4, 1024, 1024, 8
KD, VD = 512, 1024
DK, DV = 64, 128
CONV = 4
GATE_NORM = 16.0
EPS = 1e-5
LN8 = float(np.log(8.0))
RSQRT_MAGIC = 0x5F3759DF
KT_ORDER = (0, 4, 1, 5, 2, 6, 3, 7)

# per-core shapes
KDC, VDC = 256, 512          # q/k and v/gate channels per core
MIQ, MIV = 2, 4              # 128-wide channel tiles for q/k and v
C, NCH = 128, 8              # chunk length, number of chunks
G = 2                        # head groups of 2 heads (128 chans) per core
NCORES = 8
BFNP = ml_dtypes.bfloat16


def build_program():
    nc = bacc.Bacc("TRN2", target_bir_lowering=False, debug=False)

    # ---- DRAM I/O (all host-packed to partition-major contiguous) ---------
    srcT_d = nc.dram_tensor("srcT_in", [8, 128, T], BF16, kind="ExternalInput")
    wq_d = nc.dram_tensor("wq", [128, MIQ, 8, 128], BF16, kind="ExternalInput")
    wk_d = nc.dram_tensor("wk", [128, MIQ, 8, 128], BF16, kind="ExternalInput")
    wv_d = nc.dram_tensor("wv", [128, MIV, 8, 128], BF16, kind="ExternalInput")
    wgate_d = nc.dram_tensor("wgate", [128, 8, VDC], BF16, kind="ExternalInput")
    wg1_d = nc.dram_tensor("wg1", [128, 8, 16], BF16, kind="ExternalInput")
    wg2b_d = nc.dram_tensor("wg2b", [17, KDC], BF16, kind="ExternalInput")
    wo_d = nc.dram_tensor("wo", [128, MIV, D], BF16, kind="ExternalInput")
    convdiag_d = nc.dram_tensor(
        "convdiag", [128, 2 * MIQ + MIV, CONV, 128], BF16, kind="ExternalInput"
    )
    maskc_d = nc.dram_tensor("maskc", [128, NCH], F32, kind="ExternalInput")
    out_d = nc.dram_tensor("out", [T, D], F32, kind="ExternalOutput")

    ident_np = np.eye(128, dtype=np.float32)
    u = np.triu(np.ones((128, 128), np.float32))  # U[s,t] = 1 iff s <= t
    ident_d = nc.inline_tensor(ident_np, "ident_c")
    triu2_d = nc.inline_tensor(np.concatenate([u, u], axis=1), "triu2_c")

    # ---- static SBUF -------------------------------------------------------
    srcT = nc.alloc_sbuf_tensor("srcT", [128, 8, T], BF16)
    wq_sb = nc.alloc_sbuf_tensor("wq_sb", [128, MIQ, 8, 128], BF16)
    wk_sb = nc.alloc_sbuf_tensor("wk_sb", [128, MIQ, 8, 128], BF16)
    wv_sb = nc.alloc_sbuf_tensor("wv_sb", [128, MIV, 8, 128], BF16)
    wgate_sb = nc.alloc_sbuf_tensor("wgate_sb", [128, 8, VDC], BF16)
    wg1_sb = nc.alloc_sbuf_tensor("wg1_sb", [128, 8, 16], BF16)
    wg2b_sb = nc.alloc_sbuf_tensor("wg2b_sb", [17, KDC], BF16)
    wo_sb = nc.alloc_sbuf_tensor("wo_sb", [128, MIV, D], BF16)
    dgall = nc.alloc_sbuf_tensor("dgall", [128, 2 * MIQ + MIV, CONV, 128], BF16)

    pre_all = nc.alloc_sbuf_tensor("pre_all", [128, 8, 1027], BF16)
    q_sb = nc.alloc_sbuf_tensor("q_sb", [128, MIQ, T], F32R)     # silu(conv q) then q~
    k_sb = nc.alloc_sbuf_tensor("k_sb", [128, MIQ, T], F32R)     # silu(conv k) then k~
    v_sb = nc.alloc_sbuf_tensor("v_sb", [128, MIV, T], F32R)
    gate_sb = nc.alloc_sbuf_tensor("gate_sb", [128, NCH, VDC], F32R)
    xgT = nc.alloc_sbuf_tensor("xgT", [17, T], BF16)             # (src@Wg1)^T + ones row
    spT = nc.alloc_sbuf_tensor("spT", [128, MIQ, T], F32)        # softplus(-gk_logit)
    bsum = nc.alloc_sbuf_tensor("bsum", [128, MIQ, T], F32)      # per-chunk cumsum of spT
    bCn = nc.alloc_sbuf_tensor("bCn", [128, MIQ, NCH], F32)      # -spsum_last/16 per chunk
    texp_q = nc.alloc_sbuf_tensor("texp_q", [128, MIQ, T], F32)  # exp(-bsum/16)/8
    texp_k = nc.alloc_sbuf_tensor("texp_k", [128, MIQ, T], F32)  # exp(+bsum/16)
    ssq_all = nc.alloc_sbuf_tensor("ssq_all", [128, NCH * 4], F32)   # col = c*4 + head
    rrms_all = nc.alloc_sbuf_tensor("rrms_all", [128, NCH * 4], F32)
    magic_sb = nc.alloc_sbuf_tensor("magic_sb", [128, 16], I32)
    Eall = nc.alloc_sbuf_tensor("Eall", [128, MIQ, NCH], F32)    # exp(b_C) per chunk
    maskc_sb = nc.alloc_sbuf_tensor("maskc_sb", [128, NCH], F32)
    ident = nc.alloc_sbuf_tensor("ident", [128, 128], F32R)
    triu2 = nc.alloc_sbuf_tensor("triu2", [128, 256], F32)
    ones_sb = nc.alloc_sbuf_tensor("ones_sb", [128, 128], F32)
    ogT = nc.alloc_sbuf_tensor("ogT", [128, MIV, T], BF16)
    khnat_all = nc.alloc_sbuf_tensor("khnat_all", [128, MIQ, NCH, 128], F32R)
    o_sb = nc.alloc_sbuf_tensor("o_sb", [128, NCH, VDC], BF16)
    Sblk = [nc.alloc_sbuf_tensor(f"Sblk{g}", [128, 256], F32R) for g in range(G)]
    qblk = [nc.alloc_sbuf_tensor(f"qblk{g}", [128, 256], F32R) for g in range(G)]
    negln8 = nc.alloc_sbuf_tensor("negln8", [128, 1], F32)
    eps_col = nc.alloc_sbuf_tensor("eps_col", [128, 1], F32)

    with tile.TileContext(nc) as tc:
        with tc.tile_pool(name="scr", bufs=4) as scr:
            # ---- phase 0: DMAs, need-ordered and chained so late weights
            # queue BEHIND src instead of stealing HBM bandwidth beside it.
            # The sync engine is busy with Tile barrier work at startup, so
            # it only carries tensors first needed by the chunk loop.
            # scalar queue carries ONLY the tiny gk weights: DMA
            # instructions block the issuing engine, and the scalar (ACT)
            # engine's exp chain gates the whole q~/k~ path.
            nc.scalar.dma_start(
                out=wg1_sb[:], in_=wg1_d[:].rearrange("p a b -> p (a b)")
            )
            nc.scalar.dma_start(out=wg2b_sb[:], in_=wg2b_d[:])
            for kt in range(4, 8):
                nc.scalar.dma_start(out=srcT[:, kt, :], in_=srcT_d[kt])
            for kt in range(2):
                nc.gpsimd.dma_start(out=srcT[:, kt, :], in_=srcT_d[kt])
            nc.gpsimd.dma_start(
                out=wq_sb[:].rearrange("p a b c -> p (a b c)"),
                in_=wq_d[:].rearrange("p a b c -> p (a b c)"),
            )
            for kt in range(2, 4):
                nc.gpsimd.dma_start(out=srcT[:, kt, :], in_=srcT_d[kt])
            nc.gpsimd.dma_start(
                out=wk_sb[:].rearrange("p a b c -> p (a b c)"),
                in_=wk_d[:].rearrange("p a b c -> p (a b c)"),
            )
            nc.gpsimd.dma_start(
                out=wv_sb[:].rearrange("p a b c -> p (a b c)"),
                in_=wv_d[:].rearrange("p a b c -> p (a b c)"),
            )
            nc.gpsimd.dma_start(
                out=dgall[:].rearrange("p a b c -> p (a b c)"),
                in_=convdiag_d[:].rearrange("p a b c -> p (a b c)"),
            )
            nc.gpsimd.dma_start(
                out=wgate_sb[:].rearrange("p a b -> p (a b)"),
                in_=wgate_d[:].rearrange("p a b -> p (a b)"),
            )
            nc.gpsimd.dma_start(
                out=wo_sb[:].rearrange("p a b -> p (a b)"),
                in_=wo_d[:].rearrange("p a b -> p (a b)"),
            )
            nc.sync.dma_start(out=ident[:].bitcast(F32), in_=ident_d[:])
            nc.sync.dma_start(out=triu2[:], in_=triu2_d[:])
            nc.sync.dma_start(out=maskc_sb[:], in_=maskc_d[:])

            nc.vector.memset(ones_sb[:], 1.0)
            # row 16 must be ones (bias row); rows 0..15 are overwritten later
            nc.vector.memset(xgT[:], 1.0)
            nc.vector.memset(negln8[:], -LN8)
            nc.vector.memset(eps_col[:], EPS)
            nc.vector.memset(magic_sb[:], RSQRT_MAGIC)
            for g in range(G):
                nc.vector.memset(Sblk[g][:].bitcast(F32), 0.0)
                nc.vector.memset(qblk[g][:].bitcast(F32), 0.0)

            # ================= block A: gk chain + pre-activations ==========
            with (
                tc.tile_pool(name="scr2", bufs=2) as scr2,
                tc.tile_pool(name="ps_proj", bufs=6, space="PSUM") as ps_proj,
                tc.tile_pool(name="ps_cv", bufs=2, space="PSUM") as ps_cv,
            ):
                # gk logits: xg^T = (src @ Wg1)^T with an appended ones row
                for nh in range(2):
                    p = ps_proj.tile([128, 512], F32, name="pp_xg", tag="pp")
                    for ki, kt in enumerate(KT_ORDER):
                        nc.tensor.matmul(
                            p[0:16, :],
                            wg1_sb[:, kt, :],
                            srcT[:, kt, nh * 512:(nh + 1) * 512],
                            start=(ki == 0),
                            stop=(ki == 7),
                        )
                    nc.vector.tensor_copy(
                        out=xgT[0:16, nh * 512:(nh + 1) * 512], in_=p[0:16, :]
                    )
                # spT = softplus(-(xg @ Wg2 + bg2)) = log(1 + exp(-logit)).
                # All Exps before all Lns; single table set era.
                enxs = []
                for mi in range(MIQ):
                    for nh in range(2):
                        p = ps_proj.tile([128, 512], F32, name="pp_sp", tag="pp")
                        nc.tensor.matmul(
                            p[:],
                            wg2b_sb[:, mi * 128:(mi + 1) * 128],
                            xgT[:, nh * 512:(nh + 1) * 512],
                            start=True,
                            stop=True,
                        )
                        enx = scr2.tile(
                            [128, 512], F32, name="enx", tag="enx", bufs=4
                        )
                        nc.scalar.activation(enx[:], p[:], AF.Exp, scale=-1.0)
                        enxs.append((mi, nh, enx))
                for mi, nh, enx in enxs:
                    nc.scalar.activation(
                        spT[:, mi, nh * 512:(nh + 1) * 512], enx[:],
                        AF.Ln, bias=1.0,
                    )
                # per-chunk inclusive cumsum of spT; chunk-end exp factors
                for mi in range(MIQ):
                    for c in range(NCH):
                        csl = slice(c * 128, (c + 1) * 128)
                        nc.vector.tensor_tensor_scan(
                            out=bsum[:, mi, csl],
                            data0=ones_sb[:],
                            data1=spT[:, mi, csl],
                            initial=0.0,
                            op0=OP.mult,
                            op1=OP.add,
                        )
                        nc.vector.tensor_scalar_mul(
                            bCn[:, mi, c:c + 1],
                            bsum[:, mi, c * 128 + 127:c * 128 + 128],
                            -1.0 / GATE_NORM,
                        )
                    nc.scalar.activation(Eall[:, mi, :], bCn[:, mi, :], AF.Exp)
                    for nh in range(2):
                        hsl = slice(nh * 512, (nh + 1) * 512)
                        nc.scalar.activation(
                            texp_q[:, mi, hsl], bsum[:, mi, hsl], AF.Exp,
                            scale=-1.0 / GATE_NORM, bias=negln8[:],
                        )
                        nc.scalar.activation(
                            texp_k[:, mi, hsl], bsum[:, mi, hsl], AF.Exp,
                            scale=1.0 / GATE_NORM,
                        )

                # dense bf16 pre-activation matmuls; plain DVE evacuations
                def proj_pre(w_sb, mi_count, base):
                    for mi in range(mi_count):
                        for nh in range(2):
                            p = ps_proj.tile([128, 512], F32, name="pp", tag="pp")
                            for ki, kt in enumerate(KT_ORDER):
                                nc.tensor.matmul(
                                    p[:],
                                    w_sb[:, mi, kt, :],
                                    srcT[:, kt, nh * 512:(nh + 1) * 512],
                                    start=(ki == 0),
                                    stop=(ki == 7),
                                )
                            nc.vector.tensor_copy(
                                out=pre_all[:, base + mi,
                                            3 + nh * 512:3 + (nh + 1) * 512],
                                in_=p[:],
                            )

                for ti in range(8):
                    nc.gpsimd.memset(pre_all[:, ti, 0:3], 0.0)
                proj_pre(wq_sb, MIQ, 0)
                proj_pre(wk_sb, MIQ, MIQ)
                proj_pre(wv_sb, MIV, 2 * MIQ)

                # causal conv as 4 shifted diag matmuls + single-op SiLU
                # (first Silu switches the ACT table set exactly once)
                def conv_silu(base, dst, mi_count, nh):
                    for mi in range(mi_count):
                        ti = base + mi
                        cp = ps_cv.tile([128, 512], F32, name="cp", tag="cp")
                        for j in range(CONV):
                            nc.tensor.matmul(
                                cp[:],
                                dgall[:, ti, j, :],
                                pre_all[:, ti, nh * 512 + j:nh * 512 + j + 512],
                                start=(j == 0),
                                stop=(j == 3),
                            )
                        nc.scalar.activation(
                            dst[:, mi, nh * 512:(nh + 1) * 512], cp[:], AF.Silu
                        )

                for nh in range(2):
                    conv_silu(0, q_sb, MIQ, nh)
                    conv_silu(MIQ, k_sb, MIQ, nh)

                # q~ = q exp(b)/8 and k~ = k exp(-b), in place (DVE)
                for mi in range(MIQ):
                    nc.vector.tensor_mul(
                        q_sb[:, mi, :], q_sb[:, mi, :], texp_q[:, mi, :]
                    )
                    nc.vector.tensor_mul(
                        k_sb[:, mi, :], k_sb[:, mi, :], texp_k[:, mi, :]
                    )

            # ====== block B: v conv + gate proj interleaved with the GLA
            # recurrence, in two chunk-halves so the serial chunk chain
            # (DVE/ACT-paced) hides under dense PE work ======================
            def gate_proj(ps_pool, mt_range):
                for mt in mt_range:
                    p = ps_pool.tile([128, 512], F32, name="pp_gate", tag="ppg")
                    for kt in range(8):
                        nc.tensor.matmul(
                            p[:],
                            srcT[:, kt, mt * 128:(mt + 1) * 128],
                            wgate_sb[:, kt, :],
                            start=(kt == 0),
                            stop=(kt == 7),
                        )
                    nc.scalar.activation(gate_sb[:, mt, :], p[:], AF.Silu)

            def conv_silu_v(ps_pool, nh):
                for mi in range(MIV):
                    ti = 2 * MIQ + mi
                    cp = ps_pool.tile([128, 512], F32, name="cpv", tag="ppg")
                    for j in range(CONV):
                        nc.tensor.matmul(
                            cp[:],
                            dgall[:, ti, j, :],
                            pre_all[:, ti, nh * 512 + j:nh * 512 + j + 512],
                            start=(j == 0),
                            stop=(j == 3),
                        )
                    nc.scalar.activation(
                        v_sb[:, mi, nh * 512:(nh + 1) * 512], cp[:], AF.Silu
                    )

            def chunk_iter(ps_h, ps_o_pool, ps_kp, c):
                csl = slice(c * 128, (c + 1) * 128)
                for g in range(G):
                    qt = q_sb[:, g, csl]
                    kt_ = k_sb[:, g, csl]
                    e_col = Eall[:, g, c:c + 1]
                    # block-diag q for the two heads (idle GPSIMD engine)
                    nc.gpsimd.tensor_copy(out=qblk[g][0:64, 0:128], in_=qt[0:64, :])
                    nc.gpsimd.tensor_copy(
                        out=qblk[g][64:128, 128:256], in_=qt[64:128, :]
                    )
                    # A~[s, t] for both heads: (s, [t_h0 | t_h1])
                    ps_a = ps_h.tile([128, 256], F32, name="ps_a", tag="ps_h")
                    nc.tensor.matmul(
                        ps_a[:], kt_, qblk[g][:], start=True, stop=True
                    )
                    a_sb = scr.tile([128, 256], F32R, name="a_sb", tag="a_sb")
                    nc.vector.tensor_mul(a_sb[:], ps_a[:], triu2[:])
                    # v chunk, time-major; padding mask folded into the ACT
                    # evacuation as a per-partition scale
                    ps_v = ps_h.tile([128, 256], F32R, name="ps_v", tag="ps_h")
                    nc.tensor.matmul(
                        ps_v[:, 0:128], v_sb[:, 2 * g, csl], ident[:],
                        is_transpose=True, start=True, stop=False,
                        skip_group_check=True,
                    )
                    nc.tensor.matmul(
                        ps_v[:, 128:256], v_sb[:, 2 * g + 1, csl], ident[:],
                        is_transpose=True, start=False, stop=True,
                        skip_group_check=True,
                    )
                    vnat = scr.tile([128, 256], F32R, name="vnat", tag="vnat")
                    if g == 0:
                        nc.scalar.activation(
                            vnat[:], ps_v[:], AF.Copy, scale=maskc_sb[:, c:c + 1]
                        )
                    else:
                        nc.vector.tensor_scalar_mul(
                            vnat[:], ps_v[:], maskc_sb[:, c:c + 1]
                        )
                    khnat = khnat_all[:, g, c, :]
                    # o = A~^T v (intra) + q~ @ S (inter)
                    ps_o = ps_o_pool.tile([128, 256], F32, name="ps_o", tag="ps_o")
                    nc.tensor.matmul(
                        ps_o[:, 0:128], a_sb[:, 0:128], vnat[:, 0:128],
                        start=True, stop=False, skip_group_check=True,
                    )
                    nc.tensor.matmul(
                        ps_o[:, 128:256], a_sb[:, 128:256], vnat[:, 128:256],
                        start=False, stop=False, skip_group_check=True,
                    )
                    nc.tensor.matmul(
                        ps_o[:], qt, Sblk[g][:],
                        start=False, stop=True, skip_group_check=True,
                    )
                    # state update: S = diag(exp(b_C)) S + k^T v
                    ps_s = ps_h.tile([128, 256], F32, name="ps_s", tag="ps_h")
                    nc.tensor.matmul(
                        ps_s[:], khnat, vnat[:], start=True, stop=True
                    )
                    nc.vector.scalar_tensor_tensor(
                        out=Sblk[g][0:64, 0:128],
                        in0=Sblk[g][0:64, 0:128],
                        scalar=e_col[0:64, :],
                        in1=ps_s[0:64, 0:128],
                        op0=OP.mult,
                        op1=OP.add,
                    )
                    nc.vector.scalar_tensor_tensor(
                        out=Sblk[g][64:128, 128:256],
                        in0=Sblk[g][64:128, 128:256],
                        scalar=e_col[64:128, :],
                        in1=ps_s[64:128, 128:256],
                        op0=OP.mult,
                        op1=OP.add,
                    )
                    # evacuate o: per-head sums of squares (ACT, Square is in
                    # every table set), then swish gate in place (DVE)
                    for lh in range(2):
                        sqd = scr.tile([128, 128], F32, name="sqd", tag="sqd")
                        idx = c * 4 + 2 * g + lh
                        nc.scalar.activation(
                            sqd[:], ps_o[:, lh * 128:(lh + 1) * 128],
                            AF.Square,
                            accum_out=ssq_all[:, idx:idx + 1],
                        )
                    gsl = slice(g * 256, (g + 1) * 256)
                    if g == 0:
                        nc.vector.tensor_copy(out=o_sb[:, c, gsl], in_=ps_o[:])
                    else:
                        nc.scalar.copy(out=o_sb[:, c, gsl], in_=ps_o[:])

            def rsqrt_batch(lo, hi):
                # rrms = 1/sqrt(ssq/DV + eps) on the DVE: quake seed + two
                # Newton steps.  No ACT table set switch needed.
                sl = slice(lo, hi)
                n = hi - lo
                ms = scr.tile([128, n], F32, name="ms", tag="rsq")
                nc.vector.tensor_scalar(
                    out=ms[:], in0=ssq_all[:, sl],
                    scalar1=1.0 / DV, scalar2=EPS, op0=OP.mult, op1=OP.add,
                )
                ih = scr.tile([128, n], I32, name="ih", tag="rsqi")
                nc.vector.tensor_scalar(
                    out=ih[:], in0=ms[:].bitcast(I32),
                    scalar1=1, scalar2=None, op0=OP.logical_shift_right,
                )
                y = scr.tile([128, n], F32, name="y", tag="rsq2")
                nc.vector.scalar_tensor_tensor(
                    out=y[:].bitcast(I32), in0=magic_sb[:, 0:n], scalar=0,
                    in1=ih[:], op0=OP.add, op1=OP.subtract,
                )
                for it in range(2):
                    z = scr.tile([128, n], F32, name="z", tag="rsq3")
                    nc.vector.tensor_mul(z[:], y[:], y[:])
                    nc.vector.tensor_mul(z[:], z[:], ms[:])
                    nc.vector.tensor_scalar(
                        out=z[:], in0=z[:],
                        scalar1=-0.5, scalar2=1.5, op0=OP.mult, op1=OP.add,
                    )
                    dst = rrms_all[:, sl] if it == 1 else y[:]
                    nc.vector.tensor_mul(dst, y[:], z[:])

            def tail_iter(ps_out_pool, ps_gt_pool, stage_pool, c):
                csl = slice(c * 128, (c + 1) * 128)
                # gated output: gate_sb <- (o * gate) * rr  (Pool engine;
                # it has no TensorScalarPtr, so two TENSOR_TENSOR passes)
                for gg in range(2):
                    hsl = slice(gg * 256, (gg + 1) * 256)
                    rr = rrms_all[:, c * 4 + 2 * gg:c * 4 + 2 * gg + 2,
                                  None].to_broadcast((128, 2, 128))
                    nc.gpsimd.tensor_mul(
                        gate_sb[:, c, hsl], gate_sb[:, c, hsl], o_sb[:, c, hsl]
                    )
                    nc.gpsimd.tensor_mul(
                        gate_sb[:, c, hsl].rearrange("p (h x) -> p h x", h=2),
                        gate_sb[:, c, hsl].rearrange("p (h x) -> p h x", h=2),
                        rr,
                    )
                for h in range(0, 4, 2):
                    ps_g = ps_gt_pool.tile([128, 256], F32R, name="ps_g", tag="psg")
                    nc.tensor.matmul(
                        ps_g[:, 0:128], gate_sb[:, c, h * 128:(h + 1) * 128],
                        ident[:], is_transpose=True, start=True, stop=False,
                        skip_group_check=True,
                    )
                    nc.tensor.matmul(
                        ps_g[:, 128:256],
                        gate_sb[:, c, (h + 1) * 128:(h + 2) * 128],
                        ident[:], is_transpose=True, start=False, stop=True,
                        skip_group_check=True,
                    )
                    nc.vector.tensor_copy(
                        out=ogT[:, h:h + 2, csl],
                        in_=ps_g[:].rearrange("p (a b) -> p a b", a=2),
                    )
                for nh in range(2):
                    p = ps_out_pool.tile([128, 512], F32, name="p_out", tag="p_out")
                    for h in range(4):
                        nc.tensor.matmul(
                            p[:],
                            ogT[:, h, csl],
                            wo_sb[:, h, nh * 512:(nh + 1) * 512],
                            start=(h == 0),
                            stop=(h == 3),
                        )
                    stage = stage_pool.tile(
                        [128, 512], F32, name="stage", tag="stage"
                    )
                    nc.scalar.copy(out=stage[:], in_=p[:])
                    out_eng = nc.sync if nh == 0 else nc.gpsimd
                    out_eng.dma_start(
                        out=out_d[c * 128:(c + 1) * 128,
                                  nh * 512:(nh + 1) * 512],
                        in_=stage[:],
                    )

            def khat_prep(ps_kp, c, g):
                # k^ = k~ * exp(b_C), transposed to time-major, off the
                # recurrence chain (runs under the conv/gate PE window)
                csl = slice(c * 128, (c + 1) * 128)
                kh_s = scr.tile([128, 128], F32R, name="kh_s", tag="kh_s")
                nc.vector.tensor_scalar_mul(
                    kh_s[:], k_sb[:, g, csl], Eall[:, g, c:c + 1]
                )
                ps_k = ps_kp.tile([128, 128], F32R, name="ps_k", tag="psk")
                nc.tensor.transpose(ps_k[:], kh_s[:], ident[:])
                nc.scalar.copy(out=khnat_all[:, g, c, :], in_=ps_k[:])

            with (
                tc.tile_pool(name="ps_h", bufs=3, space="PSUM") as ps_h,
                tc.tile_pool(name="ps_o", bufs=2, space="PSUM") as ps_o_pool,
                tc.tile_pool(name="stage", bufs=3) as stage_pool,
            ):
                with (
                    tc.tile_pool(name="ps_pg", bufs=2, space="PSUM") as ps_pg,
                    tc.tile_pool(name="ps_k", bufs=1, space="PSUM") as ps_kp,
                ):
                    for c in range(NCH):
                        for g in range(G):
                            khat_prep(ps_kp, c, g)
                    conv_silu_v(ps_pg, 0)
                    gate_proj(ps_pg, range(0, 4))
                    for c in range(4):
                        chunk_iter(ps_h, ps_o_pool, ps_kp, c)
                    rsqrt_batch(0, 16)
                    conv_silu_v(ps_pg, 1)
                    gate_proj(ps_pg, range(4, 8))
                # pg+k banks freed: tail 0..3 (Pool/PE-heavy, light on
                # DVE/ACT) overlaps the second-half recurrence
                with (
                    tc.tile_pool(name="ps_outA", bufs=2, space="PSUM") as ps_out_a,
                    tc.tile_pool(name="ps_gtA", bufs=1, space="PSUM") as ps_gt_a,
                ):
                    for c in range(4, 8):
                        chunk_iter(ps_h, ps_o_pool, None, c)
                        rsqrt_batch(c * 4, c * 4 + 4)
                    for c in range(0, 4):
                        tail_iter(ps_out_a, ps_gt_a, stage_pool, c)

            with (
                tc.tile_pool(name="ps_outB", bufs=3, space="PSUM") as ps_out_b,
                tc.tile_pool(name="ps_gtB", bufs=2, space="PSUM") as ps_gt_b,
                tc.tile_pool(name="stage2", bufs=3) as stage_pool2,
            ):
                for c in range(4, NCH):
                    tail_iter(ps_out_b, ps_gt_b, stage_pool2, c)

    nc.compile()
    return nc


_NC_CACHE = None


def _get_program():
    global _NC_CACHE
    if _NC_CACHE is None:
        _NC_CACHE = build_program()
    return _NC_CACHE


def shard_inputs(
    src, valid_mask, Wq, Wk, Wv, conv_q_w, conv_k_w, conv_v_w,
    Wg1, Wg2, bg2, Wgate, rms_w, Wo,
):
    """Build the 8 per-core input maps (bf16, partition-major packed)."""
    f = np.float32
    src = np.asarray(src, f)
    valid_mask = np.asarray(valid_mask)
    in_maps = []
    wo_scaled = np.asarray(Wo, f) * np.tile(np.asarray(rms_w, f), VD // DV)[:, None]

    def pack_w(w):
        # [D, M] -> [128, M//128, 8, 128]: per-partition contiguous runs
        m = w.shape[1]
        return np.ascontiguousarray(
            w.reshape(8, 128, m // 128, 128).transpose(1, 2, 0, 3)
        ).astype(BFNP)

    for core in range(NCORES):
        b, hg = core // 2, core % 2
        qs = slice(hg * KDC, (hg + 1) * KDC)
        vs = slice(hg * VDC, (hg + 1) * VDC)
        wg2b = np.concatenate(
            [np.asarray(Wg2, f)[:, qs], np.asarray(bg2, f)[None, qs]], axis=0
        )

        # conv diagonal tiles: [128p, tile, tap, 128]
        conv_diag = np.zeros((2 * MIQ + MIV, CONV, 128, 128), f)
        tiles = []
        for w, sel, n in ((conv_q_w, qs, MIQ), (conv_k_w, qs, MIQ),
                          (conv_v_w, vs, MIV)):
            wa = np.asarray(w, f)[sel]
            tiles.extend(wa[i * 128:(i + 1) * 128] for i in range(n))
        for ti, wt in enumerate(tiles):      # wt: (128, 4)
            for j in range(CONV):
                np.fill_diagonal(conv_diag[ti, j], wt[:, j])
        conv_diag = np.ascontiguousarray(conv_diag.transpose(2, 0, 1, 3)).astype(BFNP)

        in_maps.append({
            "srcT_in": np.ascontiguousarray(
                src[b].T.reshape(8, 128, T)
            ).astype(BFNP),
            "wq": pack_w(np.asarray(Wq, f)[:, qs]),
            "wk": pack_w(np.asarray(Wk, f)[:, qs]),
            "wv": pack_w(np.asarray(Wv, f)[:, vs]),
            "wgate": np.ascontiguousarray(
                np.asarray(Wgate, f)[:, vs].reshape(8, 128, VDC).transpose(1, 0, 2)
            ).astype(BFNP),
            "wg1": np.ascontiguousarray(
                np.asarray(Wg1, f).reshape(8, 128, 16).transpose(1, 0, 2)
            ).astype(BFNP),
            "wg2b": np.ascontiguousarray(wg2b).astype(BFNP),
            "wo": np.ascontiguousarray(
                wo_scaled[vs, :].reshape(MIV, 128, D).transpose(1, 0, 2)
            ).astype(BFNP),
            "convdiag": conv_diag,
            "maskc": np.ascontiguousarray(
                valid_mask[b].astype(f).reshape(NCH, 128).T
            ),
        })
    return in_maps


def kernel(**inputs):
    nc = _get_program()
    in_maps = shard_inputs(**inputs)
    res = run_bass_kernel_spmd(nc, in_maps, list(range(NCORES)))
    out = np.zeros((B, T, D), np.float32)
    for core in range(NCORES):
        out[core // 2] += res.results[core]["out"]
    return out


if __name__ == "__main__":
    prog = _get_program()
    print("program built OK")
